# revision 16
# baseline (speedup 1.0000x reference)
"""GAU (Gated Attention Unit) fused kernel for Trainium2, SPMD over 8 NeuronCores.

Sharding: data-parallel over batch (B=4) x query-sequence-halves (2) = 8 cores.
Each core computes the full GAU for its (batch, query-half).

v2 design (vs the DRAM-spill baseline):
  - Host folds ln_g/ln_b into Wh/Wqk (exact: (n*g+b)@W = n@(g.*W) + b@W).
  - Host reorders each core's key rows so its query half is ALWAYS rows
    [0:SQ] -> query-column slicing of normed^T is compile-time under SPMD.
    Attention is invariant to key order; the T5 bias uses two banded tables
    (one per j-half group) to account for the reorder.
  - Two-pass LayerNorm: pass 1 computes bn_stats from bf16 cast-DMA loads;
    one batched Rsqrt covers all tiles (2 act-table switches total, vs 2
    per tile when Sqrt/Silu interleave).  Normalize runs on DVE
    (tensor_scalar with per-partition rstd/-mu*rstd scalars).
  - v ([j,h] fp8) stays SBUF-resident (8MB) -- no DRAM round trip.
  - gate is computed on the fly per (i-block, h-chunk) in the attention
    phase; its psum is [h,i] so bhg folds into the Act silu bias.
  - q/k in fp8 (x32 prescale each) -> sim matmul streams fp8.
  - attn2 = relu(s1)^2 via two DVE ops: STT(psum*c + bt) -> bf16 s1,
    STT(max(s1,0)*s1) -> fp8.
"""

import math
import os
import sys

for _p in ("/opt/trn_rl_repo", "/root/.axon_site/_ro/trn_rl_repo"):
    if os.path.isdir(_p) and _p not in sys.path:
        sys.path.append(_p)

import numpy as np
import ml_dtypes

import concourse.bass as bass
import concourse.tile as tile
from concourse import mybir
from concourse.bass_utils import run_bass_kernel_spmd
from concourse.masks import make_identity

# Problem dims (hardcoded per spec)
B, S, D, QK, H = 4, 4096, 1024, 128, 2048
NUM_BUCKETS, MAX_DIST = 32, 128
LN_EPS = 1e-5
N_CORES = 8

P = 128
NB = 512  # free-dim block for matmuls

BF16 = mybir.dt.bfloat16
FP8 = mybir.dt.float8e4
F32 = mybir.dt.float32

AQ = 32.0   # q fp8 prescale
AK = 32.0   # k fp8 prescale
BT_SCALE = 1024.0                    # bias-table prescale (s1 = 1024*(sim+bias)/S)
SIM_DESCALE = BT_SCALE / (S * AQ * AK)
ATTN_DESCALE = 1.0 / (BT_SCALE * BT_SCALE)

_NC_CACHE = {}


def _split_excess_waits(nc, max_waits=1):
    """This container's walrus rejects instructions carrying more than one
    sem wait ("Too many sync wait commands").  Move excess waits onto
    same-engine nops inserted immediately before the instruction — engine
    FIFO order makes that semantically identical."""
    f = nc.m.functions[0]
    for bb in list(f.blocks):
        il = list(bb.instructions)
        out = []
        changed = False
        for inst in il:
            si = inst.sync_info
            if si is not None and si.on_wait and len(si.on_wait) > max_waits:
                waits = list(si.on_wait)
                moved, keep = waits[:-max_waits], waits[-max_waits:]
                si.on_wait = keep
                for w in moved:
                    eng = nc.engines[inst.engine]
                    cur_bb = nc.cur_bb.bb
                    n_before = len(cur_bb.instructions)
                    nop = eng.nop()
                    # pop the freshly appended nop from wherever it landed
                    tail = list(cur_bb.instructions)
                    assert tail[-1] is nop.ins and len(tail) == n_before + 1
                    cur_bb.instructions = tail[:-1]
                    nsi = nop.ins.sync_info
                    if nsi is None:
                        nop.ins.sync_info = mybir.SyncInfo(
                            on_wait=[w], on_update=[])
                    else:
                        nsi.on_wait = [w]
                    out.append(nop.ins)
                changed = True
            out.append(inst)
        if changed:
            bb.instructions = out


def _install_drain_wait_split():
    """The walrus build in this container rejects >1 sem wait on the Tile
    epilogue Drain ("Too many sync wait commands").  Split the extra waits
    onto explicit SP nops (they only need to precede the final barrier)."""
    from concourse.vector_clock import ScopedClock

    if getattr(tile.TileContext, "_drain_split_installed", False):
        return

    def _patched(self, tick_clock, wait_clock):
        drain_inst = self.nc.sync.drain()
        wait_clock.add_sem_waits(
            drain_inst.ins, ScopedClock({None: tick_clock.global_clock}))
        si = drain_inst.ins.sync_info
        if si is not None and si.on_wait and len(si.on_wait) > 1:
            extra = list(si.on_wait)[1:]
            si.on_wait = [si.on_wait[0]]
            for w in extra:
                nop = self.nc.sync.nop()
                nsi = nop.ins.sync_info
                if nsi is None:
                    nop.ins.sync_info = mybir.SyncInfo(on_wait=[w], on_update=[])
                else:
                    nsi.on_wait = [w]
        self.nc.all_engine_barrier()
        assert self.sems is not None
        popped = self.nc._tile_sem_poison_stack.pop()
        assert popped is self._sem_poison
        self.nc.clear_and_free_semaphores(list(self.sems.allocated().values()))
        self.nc.all_engine_barrier()

    tile.TileContext._drain_and_barrier = _patched
    tile.TileContext._drain_split_installed = True


_install_drain_wait_split()


def build_gau_nc(S=S, SQ=S // 2, D=D, QK=QK, H=H, reps=1, use_dr=True):
    DR = 2 if use_dr else 1
    PM = mybir.MatmulPerfMode.DoubleRow if use_dr else None
    assert D % P == 0 and H % P == 0 and S % NB == 0 and SQ % NB == 0
    assert QK == P
    KD = D // P      # d chunks (8)
    NSK = S // P     # key-side seq tiles (32)
    SBK = S // NB    # key-side 512-blocks (8)
    IB = SQ // NB    # query-side 512-blocks (4)
    HC = H // P      # h 128-chunks (16)
    HB = H // NB     # h 512-blocks (4)
    JC = S // P      # j chunks (32)
    JH = JC // 2     # j tiles per half group (16)
    DB = D // NB     # output d blocks (2)
    ISUB = NB // P   # i subtiles per i-block (4)
    WT = (SQ - P) + SQ   # per-group bias table width (3968)

    nc = bass.Bass("TRN2", target_bir_lowering=False, debug=False)

    # ---- DRAM I/O ----
    xk = nc.dram_tensor("xk", [S, D], F32, kind="ExternalInput").ap()
    whv = nc.dram_tensor("whv", [D, H], FP8, kind="ExternalInput").ap()
    whg = nc.dram_tensor("whg", [D, H], FP8, kind="ExternalInput").ap()
    wqk = nc.dram_tensor("wqk", [D, QK], FP8, kind="ExternalInput").ap()
    wo = nc.dram_tensor("wo", [H, D], FP8, kind="ExternalInput").ap()
    bqk = nc.dram_tensor("bqk", [QK], F32, kind="ExternalInput").ap()
    g0 = nc.dram_tensor("g0", [QK], F32, kind="ExternalInput").ap()  # gamma0*AQ
    b0 = nc.dram_tensor("b0", [QK], F32, kind="ExternalInput").ap()  # beta0*AQ
    g1 = nc.dram_tensor("g1", [QK], F32, kind="ExternalInput").ap()  # gamma1*AK
    b1 = nc.dram_tensor("b1", [QK], F32, kind="ExternalInput").ap()  # beta1*AK
    bhv = nc.dram_tensor("bhv", [H], F32, kind="ExternalInput").ap()
    bhg = nc.dram_tensor("bhg", [H], F32, kind="ExternalInput").ap()
    bo = nc.dram_tensor("bo", [D], F32, kind="ExternalInput").ap()
    bt0 = nc.dram_tensor("bt0", [P, WT], BF16, kind="ExternalInput").ap()
    bt1 = nc.dram_tensor("bt1", [P, WT], BF16, kind="ExternalInput").ap()
    out = nc.dram_tensor("out", [SQ, D], F32, kind="ExternalOutput").ap()

    with tile.TileContext(nc) as tc:
        for _rep in range(reps):
            from contextlib import ExitStack

            with ExitStack() as outer:
                singles = outer.enter_context(tc.tile_pool(name="singles", bufs=1))
                persist = outer.enter_context(tc.tile_pool(name="persist", bufs=1))
                ps_mm = outer.enter_context(
                    tc.tile_pool(name="ps_mm", bufs=2, space="PSUM"))

                ident = singles.tile([P, P], BF16)
                make_identity(nc, ident)

                eps_sb = singles.tile([P, 1], F32)
                nc.vector.memset(eps_sb, LN_EPS)

                # small parameter tiles
                bqk_sb = singles.tile([P, 1], F32)
                nc.scalar.dma_start(bqk_sb, bqk.unsqueeze(1))
                g0_sb = singles.tile([P, 1], F32)
                nc.scalar.dma_start(g0_sb, g0.unsqueeze(1))
                b0_sb = singles.tile([P, 1], F32)
                nc.scalar.dma_start(b0_sb, b0.unsqueeze(1))
                g1_sb = singles.tile([P, 1], F32)
                nc.scalar.dma_start(g1_sb, g1.unsqueeze(1))
                b1_sb = singles.tile([P, 1], F32)
                nc.scalar.dma_start(b1_sb, b1.unsqueeze(1))
                bhg_sb = singles.tile([P, HC], F32)
                nc.scalar.dma_start(bhg_sb, bhg.rearrange("(o p) -> p o", p=P))
                bo_sb = singles.tile([P, D], BF16)
                nc.gpsimd.dma_start(bo_sb, bo.unsqueeze(0).to_broadcast((P, D)))

                wqk_sb = singles.tile([P, KD, QK], FP8)
                nc.scalar.dma_start(wqk_sb, wqk.rearrange("(o p) q -> p o q", p=P))

                # persistent big tensors
                vsb = persist.tile([P, NSK, H], FP8, tag="vsb")  # v [j, h]
                kT = persist.tile([P, S], BF16, tag="kT")
                qT = persist.tile([P, SQ], BF16, tag="qT")
                ntq = persist.tile([P, KD, SQ], FP8, tag="ntq")  # query cols

                # ---------- Phase 0/1: LN stats + normalize + projections --
                GT = 4
                NGRP = NSK // GT
                with ExitStack() as ph1:
                    xbfp = ph1.enter_context(tc.tile_pool(name="xbfp", bufs=4))
                    statp = ph1.enter_context(tc.tile_pool(name="statp", bufs=4))
                    work = ph1.enter_context(tc.tile_pool(name="work", bufs=3))
                    nrmp = ph1.enter_context(tc.tile_pool(name="nrmp",
                                                          bufs=GT + 1))
                    qwork = ph1.enter_context(tc.tile_pool(name="qwork", bufs=3))
                    ps_tr = ph1.enter_context(
                        tc.tile_pool(name="ps_tr", bufs=2, space="PSUM"))
                    ntk = ph1.enter_context(
                        tc.tile_pool(name="ntkp", bufs=1)).tile(
                            [P, KD, S], FP8, tag="ntk")

                    mva = singles.tile([P, NSK, 2], F32)
                    rstd = singles.tile([P, NSK], F32)
                    nmu = singles.tile([P, NSK], F32)

                    def emit_stats(t):
                        xbf = xbfp.tile([P, D], BF16, tag="xbf")
                        nc.gpsimd.dma_start(xbf, xk[t * P:(t + 1) * P, :])
                        stats = statp.tile([P, 2, 6], F32, tag="st")
                        for i in range(2):
                            nc.vector.bn_stats(
                                out=stats[:, i, :],
                                in_=xbf[:, i * 512:(i + 1) * 512])
                        nc.vector.bn_aggr(out=mva[:, t, :], in_=stats)

                    def emit_rstd(lo, hi):
                        """batched rstd/-mu*rstd for tiles [lo, hi)."""
                        nc.scalar.activation(
                            out=rstd[:, lo:hi], in_=mva[:, lo:hi, 1],
                            func=mybir.ActivationFunctionType.Sqrt,
                            bias=eps_sb, scale=1.0)
                        nc.vector.reciprocal(out=rstd[:, lo:hi],
                                             in_=rstd[:, lo:hi])
                        nc.vector.tensor_mul(nmu[:, lo:hi], mva[:, lo:hi, 0],
                                             rstd[:, lo:hi])
                        nc.scalar.mul(nmu[:, lo:hi], nmu[:, lo:hi], -1.0)

                    whv_sb = ph1.enter_context(
                        tc.tile_pool(name="whvp", bufs=1)).tile(
                            [P, KD, H], FP8, tag="whv")
                    nc.scalar.dma_start(
                        whv_sb, whv.rearrange("(o p) h -> p o h", p=P))
                    bhv_sb = ph1.enter_context(
                        tc.tile_pool(name="bhvp", bufs=1)).tile(
                            [1, H], BF16, tag="bhv")
                    nc.gpsimd.dma_start(bhv_sb, bhv.unsqueeze(0))
                    ones_sb = singles.tile([1, P], BF16)
                    nc.vector.memset(ones_sb, 1.0)

                    def v_proj_tile(st):
                        """v rows for key tile st -> vsb[:, st, :].  The bias
                        bhv rides the psum as a rank-1 ones-row matmul."""
                        for hb in range(HB):
                            ps = ps_mm.tile([P, NB], F32, tag="mm")
                            nc.tensor.matmul(
                                ps, ones_sb,
                                bhv_sb[:, hb * NB:(hb + 1) * NB],
                                start=True, stop=False)
                            for k in range(0, KD, DR):
                                nc.tensor.matmul(
                                    ps, ntk[:, k:k + DR, st * P:(st + 1) * P],
                                    whv_sb[:, k:k + DR, hb * NB:(hb + 1) * NB],
                                    start=False, stop=(k == KD - DR),
                                    perf_mode=PM)
                            nc.scalar.activation(
                                out=vsb[:, st, hb * NB:(hb + 1) * NB],
                                in_=ps,
                                func=mybir.ActivationFunctionType.Silu)

                    def qk_proj_block(sb):
                        """kT (and qT when sb < IB) for 512-col block sb."""
                        ps = ps_mm.tile([P, NB], F32, tag="mm")
                        for k in range(0, KD, DR):
                            nc.tensor.matmul(
                                ps, wqk_sb[:, k:k + DR, :],
                                ntk[:, k:k + DR, sb * NB:(sb + 1) * NB],
                                start=(k == 0), stop=(k == KD - DR),
                                perf_mode=PM)
                        tmp = qwork.tile([P, NB], BF16, tag="qtmp")
                        nc.scalar.activation(
                            out=tmp, in_=ps,
                            func=mybir.ActivationFunctionType.Silu,
                            bias=bqk_sb, scale=1.0)
                        nc.vector.tensor_scalar(
                            out=kT[:, sb * NB:(sb + 1) * NB],
                            in0=tmp, scalar1=g1_sb, scalar2=b1_sb,
                            op0=mybir.AluOpType.mult,
                            op1=mybir.AluOpType.add)
                        if sb < IB:
                            nc.vector.tensor_scalar(
                                out=qT[:, sb * NB:(sb + 1) * NB],
                                in0=tmp, scalar1=g0_sb, scalar2=b0_sb,
                                op0=mybir.AluOpType.mult,
                                op1=mybir.AluOpType.add)

                    # stats for the first half of tiles, then its rstd batch
                    HALF = NSK // 2
                    for t in range(HALF):
                        emit_stats(t)
                    emit_rstd(0, HALF)

                    for g in range(NGRP):
                        if g == NGRP // 2:
                            emit_rstd(HALF, NSK)
                        nrms = []
                        for tt in range(GT):
                            t = g * GT + tt
                            x_t = work.tile([P, D], F32, tag="xt")
                            nc.sync.dma_start(x_t, xk[t * P:(t + 1) * P, :])
                            nrm = nrmp.tile([P, D], BF16, tag="nrm",
                                            name=f"nrm{tt}")
                            nc.vector.tensor_scalar(
                                out=nrm, in0=x_t,
                                scalar1=rstd[:, t:t + 1],
                                scalar2=nmu[:, t:t + 1],
                                op0=mybir.AluOpType.mult,
                                op1=mybir.AluOpType.add)
                            nrms.append(nrm)
                            # interleave second-half stats into first half
                            if g < NGRP // 2:
                                emit_stats(HALF + g * GT + tt)
                        for k in range(KD):
                            pst = ps_tr.tile([P, GT, P], BF16, tag="pst")
                            for tt in range(GT):
                                nc.tensor.transpose(
                                    pst[:, tt, :],
                                    nrms[tt][:, k * P:(k + 1) * P], ident)
                            if k % 2 == 0:
                                nc.scalar.copy(
                                    out=ntk[:, k, g * GT * P:(g + 1) * GT * P],
                                    in_=pst)
                            else:
                                nc.vector.tensor_copy(
                                    out=ntk[:, k, g * GT * P:(g + 1) * GT * P],
                                    in_=pst)
                        # projections for the 4 tiles just transposed
                        for tt in range(GT):
                            v_proj_tile(g * GT + tt)
                        qk_proj_block(g)  # 8 groups == 8 kT blocks
                    nc.sync.dma_start(ntq, ntk[:, :, 0:SQ])

                # ---------- Phase 3: attention + gating + out-proj ----------
                with ExitStack() as ph3:
                    wp3 = ph3.enter_context(tc.tile_pool(name="wp3", bufs=1))
                    whg_sb = wp3.tile([P, KD, H], FP8, tag="whg")
                    nc.scalar.dma_start(
                        whg_sb, whg.rearrange("(o p) h -> p o h", p=P))
                    wo_sb = wp3.tile([P, HC, D], FP8, tag="wo")
                    nc.scalar.dma_start(
                        wo_sb, wo.rearrange("(o p) d -> p o d", p=P))
                    bt_sb = wp3.tile([P, 2, WT], BF16, tag="bt")
                    nc.scalar.dma_start(bt_sb[:, 0, :], bt0)
                    nc.scalar.dma_start(bt_sb[:, 1, :], bt1)

                    a2pool = ph3.enter_context(tc.tile_pool(name="a2p", bufs=2))
                    s1pool = ph3.enter_context(tc.tile_pool(name="s1p", bufs=2))
                    gtpool = ph3.enter_context(tc.tile_pool(name="gtp", bufs=2))
                    gopool = ph3.enter_context(tc.tile_pool(name="gop", bufs=2))
                    pspool = ph3.enter_context(tc.tile_pool(name="psp", bufs=2))
                    opool = ph3.enter_context(tc.tile_pool(name="op", bufs=2))
                    ps_sim = ph3.enter_context(
                        tc.tile_pool(name="ps_sim", bufs=2, space="PSUM"))
                    ps_gate = ph3.enter_context(
                        tc.tile_pool(name="ps_gate", bufs=2, space="PSUM"))
                    ps_acc = ph3.enter_context(
                        tc.tile_pool(name="ps_acc", bufs=2, space="PSUM"))

                    def emit_sim(ib, j):
                        """sim psum (bias via identity-matmul) -> attn2 tile."""
                        ps = ps_sim.tile([P, NB], F32, tag="sim")
                        grp = 0 if j < JH else 1
                        jl = j - JH * grp
                        m0 = ib * NB - jl * P + (SQ - P)
                        nc.tensor.matmul(
                            ps, ident, bt_sb[:, grp, m0:m0 + NB],
                            start=True, stop=False)
                        nc.tensor.matmul(
                            ps, kT[:, j * P:(j + 1) * P],
                            qT[:, ib * NB:(ib + 1) * NB],
                            start=False, stop=True)
                        rl = s1pool.tile([P, NB], BF16, tag="s1")
                        nc.vector.tensor_relu(out=rl, in_=ps)
                        nc.scalar.activation(
                            out=attn2s[ib % 2][:, j, :], in_=rl,
                            func=mybir.ActivationFunctionType.Square)

                    attn2s = [a2pool.tile([P, JC, NB], FP8, tag="attn2",
                                          name=f"attn2_{i}") for i in range(2)]
                    for j in range(JC):
                        emit_sim(0, j)
                    for ib in range(IB):
                        attn2 = attn2s[ib % 2]
                        goT = gopool.tile([P, HC, NB], FP8, tag="goT")
                        for hc in range(HC):
                            # gate psum [h, i]
                            gps = ps_gate.tile([P, NB], F32, tag="g")
                            for k in range(0, KD, DR):
                                nc.tensor.matmul(
                                    gps,
                                    whg_sb[:, k:k + DR, hc * P:(hc + 1) * P],
                                    ntq[:, k:k + DR, ib * NB:(ib + 1) * NB],
                                    start=(k == 0), stop=(k == KD - DR),
                                    perf_mode=PM)
                            gt = gtpool.tile([P, NB], BF16, tag="gt")
                            nc.scalar.activation(
                                out=gt, in_=gps,
                                func=mybir.ActivationFunctionType.Silu,
                                bias=bhg_sb[:, hc:hc + 1], scale=1.0)
                            # attention accumulation psum [h, i]
                            pacc = ps_acc.tile([P, NB], F32, tag="pacc")
                            for j in range(0, JC, DR):
                                nc.tensor.matmul(
                                    pacc,
                                    vsb[:, j:j + DR, hc * P:(hc + 1) * P],
                                    attn2[:, j:j + DR, :],
                                    start=(j == 0), stop=(j == JC - DR),
                                    perf_mode=PM)
                            nc.vector.tensor_mul(goT[:, hc, :], pacc, gt)
                            # interleave next i-block's sim pipeline
                            if ib + 1 < IB:
                                emit_sim(ib + 1, 2 * hc)
                                emit_sim(ib + 1, 2 * hc + 1)

                        # --- out projection + bias + residual ---
                        for isub in range(ISUB):
                            i0 = ib * NB + isub * P
                            xt = opool.tile([P, D], F32, tag="xres")
                            nc.sync.dma_start(xt, xk[i0:i0 + P, :])
                            for db in range(DB):
                                ps = ps_mm.tile([P, NB], F32, tag="mm")
                                for hc in range(0, HC, DR):
                                    nc.tensor.matmul(
                                        ps, goT[:, hc:hc + DR,
                                                isub * P:(isub + 1) * P],
                                        wo_sb[:, hc:hc + DR,
                                              db * NB:(db + 1) * NB],
                                        start=(hc == 0), stop=(hc == HC - DR),
                                        perf_mode=PM)
                                ot = opool.tile([P, NB], F32, tag="ot")
                                nc.vector.scalar_tensor_tensor(
                                    out=ot, in0=ps, scalar=ATTN_DESCALE,
                                    in1=bo_sb[:, db * NB:(db + 1) * NB],
                                    op0=mybir.AluOpType.mult,
                                    op1=mybir.AluOpType.add)
                                nc.gpsimd.tensor_add(
                                    ot, ot, xt[:, db * NB:(db + 1) * NB])
                                nc.sync.dma_start(
                                    out[i0:i0 + P, db * NB:(db + 1) * NB], ot)

    _split_excess_waits(nc)
    return nc


def _t5_bias_vec(rel_emb, S_, D_):
    """bv[r + S_-1] = bias for rel = k_pos - q_pos = r, scaled sqrt(D)/S."""
    r = np.arange(-(S_ - 1), S_, dtype=np.int64)
    n = (-r).astype(np.int64)
    nb = NUM_BUCKETS // 2
    me = nb // 2
    ret = (n < 0).astype(np.int64) * nb
    na = np.abs(n)
    val_large = me + (
        np.log(np.maximum(na, 1).astype(np.float32) / me)
        / math.log(MAX_DIST / me) * (nb - me)).astype(np.int64)
    val_large = np.minimum(val_large, nb - 1)
    bucket = ret + np.where(na < me, na, val_large)
    return (rel_emb[bucket, 0].astype(np.float64)
            * (float(D_) ** 0.5) / float(S_)).astype(np.float32)


def make_core_inputs(inputs, S_=S, SQ_=None, D_=D, QK_=QK, H_=H,
                     n_cores=N_CORES):
    """Build per-core in_maps from the full (unsharded) input dict."""
    if SQ_ is None:
        SQ_ = S_ // 2
    bf = ml_dtypes.bfloat16
    f8 = ml_dtypes.float8_e4m3fn
    x = np.asarray(inputs["x"], np.float32)
    Wh = np.asarray(inputs["Wh"], np.float64)
    bh = np.asarray(inputs["bh"], np.float64)
    Wqk = np.asarray(inputs["Wqk"], np.float64)
    bqk_ = np.asarray(inputs["bqk"], np.float64)
    osg = np.asarray(inputs["os_gamma"], np.float32)
    osb = np.asarray(inputs["os_beta"], np.float32)
    Wo = np.asarray(inputs["Wo"], np.float32)
    bo_ = np.asarray(inputs["bo"], np.float32)
    rel_emb = np.asarray(inputs["rel_emb"], np.float32)
    lng_ = np.asarray(inputs["ln_g"], np.float64)
    lnb_ = np.asarray(inputs["ln_b"], np.float64)

    # fold LN affine into the projection weights (exact algebra)
    Whf = lng_[:, None] * Wh
    bhf = bh + lnb_ @ Wh
    Wqkf = lng_[:, None] * Wqk
    bqkf = bqk_ + lnb_ @ Wqk

    bv = _t5_bias_vec(rel_emb, S_, D_)
    WT_ = (SQ_ - P) + SQ_
    halves = S_ // SQ_

    shared = dict(
        whv=np.ascontiguousarray(Whf[:, :H_]).astype(np.float32).astype(f8),
        whg=np.ascontiguousarray(Whf[:, H_:]).astype(np.float32).astype(f8),
        wqk=np.ascontiguousarray(Wqkf).astype(np.float32).astype(f8),
        wo=np.ascontiguousarray(Wo).astype(f8),
        bqk=bqkf.astype(np.float32),
        g0=(osg[0] * BT_SCALE / S_).astype(np.float32),
        b0=(osb[0] * BT_SCALE / S_).astype(np.float32),
        g1=osg[1].astype(np.float32),
        b1=osb[1].astype(np.float32),
        bhv=np.ascontiguousarray(bhf[:H_]).astype(np.float32),
        bhg=np.ascontiguousarray(bhf[H_:]).astype(np.float32),
        bo=bo_,
    )

    pp = np.arange(P)[:, None]
    cc = np.arange(WT_)[None, :]
    in_maps = []
    for c in range(n_cores):
        b = c // halves
        off = (c % halves) * SQ_
        other = SQ_ - off
        # reordered keys: own query half first
        xc = np.concatenate([x[b, off:off + SQ_], x[b, other:other + SQ_]],
                            axis=0)
        m = dict(shared)
        m["xk"] = np.ascontiguousarray(xc)
        # banded bias tables: bt_g[p, c] = bv[p - c + (SQ-P) + offj_g - offq
        #                                     + (S-1)] * BT_SCALE
        for gname, offj in (("bt0", off), ("bt1", other)):
            idx = pp - cc + (SQ_ - P) + offj - off + (S_ - 1)
            m[gname] = (bv[idx] * BT_SCALE).astype(bf)
        in_maps.append(m)
    return in_maps


def run_with_results(inputs, trace=False):
    key = (S, S // 2, D, QK, H)
    if key not in _NC_CACHE:
        _NC_CACHE[key] = build_gau_nc(*key)
    nc = _NC_CACHE[key]
    in_maps = make_core_inputs(inputs)
    res = run_bass_kernel_spmd(nc, in_maps, core_ids=list(range(N_CORES)),
                               trace=trace)
    SQ_ = S // 2
    halves = S // SQ_
    out = np.empty((B, S, D), np.float32)
    for c in range(N_CORES):
        b = c // halves
        off = (c % halves) * SQ_
        out[b, off:off + SQ_, :] = res.results[c]["out"]
    return out, res


def kernel(**inputs):
    return run_with_results(inputs, trace=False)[0]


# revision 17
# speedup vs baseline: 1.0005x; 1.0005x over previous
"""GAU (Gated Attention Unit) fused kernel for Trainium2, SPMD over 8 NeuronCores.

Sharding: data-parallel over batch (B=4) x query-sequence-halves (2) = 8 cores.
Each core computes the full GAU for its (batch, query-half).

v2 design (vs the DRAM-spill baseline):
  - Host folds ln_g/ln_b into Wh/Wqk (exact: (n*g+b)@W = n@(g.*W) + b@W).
  - Host reorders each core's key rows so its query half is ALWAYS rows
    [0:SQ] -> query-column slicing of normed^T is compile-time under SPMD.
    Attention is invariant to key order; the T5 bias uses two banded tables
    (one per j-half group) to account for the reorder.
  - Two-pass LayerNorm: pass 1 computes bn_stats from bf16 cast-DMA loads;
    one batched Rsqrt covers all tiles (2 act-table switches total, vs 2
    per tile when Sqrt/Silu interleave).  Normalize runs on DVE
    (tensor_scalar with per-partition rstd/-mu*rstd scalars).
  - v ([j,h] fp8) stays SBUF-resident (8MB) -- no DRAM round trip.
  - gate is computed on the fly per (i-block, h-chunk) in the attention
    phase; its psum is [h,i] so bhg folds into the Act silu bias.
  - q/k in fp8 (x32 prescale each) -> sim matmul streams fp8.
  - attn2 = relu(s1)^2 via two DVE ops: STT(psum*c + bt) -> bf16 s1,
    STT(max(s1,0)*s1) -> fp8.
"""

import math
import os
import sys

for _p in ("/opt/trn_rl_repo", "/root/.axon_site/_ro/trn_rl_repo"):
    if os.path.isdir(_p) and _p not in sys.path:
        sys.path.append(_p)

import numpy as np
import ml_dtypes

import concourse.bass as bass
import concourse.tile as tile
from concourse import mybir
from concourse.bass_utils import run_bass_kernel_spmd
from concourse.masks import make_identity

# Problem dims (hardcoded per spec)
B, S, D, QK, H = 4, 4096, 1024, 128, 2048
NUM_BUCKETS, MAX_DIST = 32, 128
LN_EPS = 1e-5
N_CORES = 8

P = 128
NB = 512  # free-dim block for matmuls

BF16 = mybir.dt.bfloat16
FP8 = mybir.dt.float8e4
F32 = mybir.dt.float32

AQ = 32.0   # q fp8 prescale
AK = 32.0   # k fp8 prescale
BT_SCALE = 1024.0                    # bias-table prescale (s1 = 1024*(sim+bias)/S)
SIM_DESCALE = BT_SCALE / (S * AQ * AK)
ATTN_DESCALE = 1.0 / (BT_SCALE * BT_SCALE)

_NC_CACHE = {}


def _split_excess_waits(nc, max_waits=1):
    """This container's walrus rejects instructions carrying more than one
    sem wait ("Too many sync wait commands").  Move excess waits onto
    same-engine nops inserted immediately before the instruction — engine
    FIFO order makes that semantically identical."""
    f = nc.m.functions[0]
    for bb in list(f.blocks):
        il = list(bb.instructions)
        out = []
        changed = False
        for inst in il:
            si = inst.sync_info
            if si is not None and si.on_wait and len(si.on_wait) > max_waits:
                waits = list(si.on_wait)
                moved, keep = waits[:-max_waits], waits[-max_waits:]
                si.on_wait = keep
                for w in moved:
                    eng = nc.engines[inst.engine]
                    cur_bb = nc.cur_bb.bb
                    n_before = len(cur_bb.instructions)
                    nop = eng.nop()
                    # pop the freshly appended nop from wherever it landed
                    tail = list(cur_bb.instructions)
                    assert tail[-1] is nop.ins and len(tail) == n_before + 1
                    cur_bb.instructions = tail[:-1]
                    nsi = nop.ins.sync_info
                    if nsi is None:
                        nop.ins.sync_info = mybir.SyncInfo(
                            on_wait=[w], on_update=[])
                    else:
                        nsi.on_wait = [w]
                    out.append(nop.ins)
                changed = True
            out.append(inst)
        if changed:
            bb.instructions = out


def _install_drain_wait_split():
    """The walrus build in this container rejects >1 sem wait on the Tile
    epilogue Drain ("Too many sync wait commands").  Split the extra waits
    onto explicit SP nops (they only need to precede the final barrier)."""
    from concourse.vector_clock import ScopedClock

    if getattr(tile.TileContext, "_drain_split_installed", False):
        return

    def _patched(self, tick_clock, wait_clock):
        drain_inst = self.nc.sync.drain()
        wait_clock.add_sem_waits(
            drain_inst.ins, ScopedClock({None: tick_clock.global_clock}))
        si = drain_inst.ins.sync_info
        if si is not None and si.on_wait and len(si.on_wait) > 1:
            extra = list(si.on_wait)[1:]
            si.on_wait = [si.on_wait[0]]
            for w in extra:
                nop = self.nc.sync.nop()
                nsi = nop.ins.sync_info
                if nsi is None:
                    nop.ins.sync_info = mybir.SyncInfo(on_wait=[w], on_update=[])
                else:
                    nsi.on_wait = [w]
        self.nc.all_engine_barrier()
        assert self.sems is not None
        popped = self.nc._tile_sem_poison_stack.pop()
        assert popped is self._sem_poison
        self.nc.clear_and_free_semaphores(list(self.sems.allocated().values()))
        self.nc.all_engine_barrier()

    tile.TileContext._drain_and_barrier = _patched
    tile.TileContext._drain_split_installed = True


_install_drain_wait_split()


def build_gau_nc(S=S, SQ=S // 2, D=D, QK=QK, H=H, reps=1, use_dr=True):
    DR = 2 if use_dr else 1
    PM = mybir.MatmulPerfMode.DoubleRow if use_dr else None
    assert D % P == 0 and H % P == 0 and S % NB == 0 and SQ % NB == 0
    assert QK == P
    KD = D // P      # d chunks (8)
    NSK = S // P     # key-side seq tiles (32)
    SBK = S // NB    # key-side 512-blocks (8)
    IB = SQ // NB    # query-side 512-blocks (4)
    HC = H // P      # h 128-chunks (16)
    HB = H // NB     # h 512-blocks (4)
    JC = S // P      # j chunks (32)
    JH = JC // 2     # j tiles per half group (16)
    DB = D // NB     # output d blocks (2)
    ISUB = NB // P   # i subtiles per i-block (4)
    WT = (SQ - P) + SQ   # per-group bias table width (3968)

    nc = bass.Bass("TRN2", target_bir_lowering=False, debug=False)

    # ---- DRAM I/O ----
    xk = nc.dram_tensor("xk", [S, D], F32, kind="ExternalInput").ap()
    whv = nc.dram_tensor("whv", [D, H], FP8, kind="ExternalInput").ap()
    whg = nc.dram_tensor("whg", [D, H], FP8, kind="ExternalInput").ap()
    wqk = nc.dram_tensor("wqk", [D, QK], FP8, kind="ExternalInput").ap()
    wo = nc.dram_tensor("wo", [H, D], FP8, kind="ExternalInput").ap()
    bqk = nc.dram_tensor("bqk", [QK], F32, kind="ExternalInput").ap()
    g0 = nc.dram_tensor("g0", [QK], F32, kind="ExternalInput").ap()  # gamma0*AQ
    b0 = nc.dram_tensor("b0", [QK], F32, kind="ExternalInput").ap()  # beta0*AQ
    g1 = nc.dram_tensor("g1", [QK], F32, kind="ExternalInput").ap()  # gamma1*AK
    b1 = nc.dram_tensor("b1", [QK], F32, kind="ExternalInput").ap()  # beta1*AK
    bhv = nc.dram_tensor("bhv", [H], F32, kind="ExternalInput").ap()
    bhg = nc.dram_tensor("bhg", [H], F32, kind="ExternalInput").ap()
    bo = nc.dram_tensor("bo", [D], F32, kind="ExternalInput").ap()
    bt0 = nc.dram_tensor("bt0", [P, WT], BF16, kind="ExternalInput").ap()
    bt1 = nc.dram_tensor("bt1", [P, WT], BF16, kind="ExternalInput").ap()
    out = nc.dram_tensor("out", [SQ, D], F32, kind="ExternalOutput").ap()

    with tile.TileContext(nc) as tc:
        for _rep in range(reps):
            from contextlib import ExitStack

            with ExitStack() as outer:
                singles = outer.enter_context(tc.tile_pool(name="singles", bufs=1))
                persist = outer.enter_context(tc.tile_pool(name="persist", bufs=1))
                ps_mm = outer.enter_context(
                    tc.tile_pool(name="ps_mm", bufs=2, space="PSUM"))

                ident = singles.tile([P, P], BF16)
                make_identity(nc, ident)

                eps_sb = singles.tile([P, 1], F32)
                nc.vector.memset(eps_sb, LN_EPS)

                # small parameter tiles
                bqk_sb = singles.tile([P, 1], F32)
                nc.scalar.dma_start(bqk_sb, bqk.unsqueeze(1))
                g0_sb = singles.tile([P, 1], F32)
                nc.scalar.dma_start(g0_sb, g0.unsqueeze(1))
                b0_sb = singles.tile([P, 1], F32)
                nc.scalar.dma_start(b0_sb, b0.unsqueeze(1))
                g1_sb = singles.tile([P, 1], F32)
                nc.scalar.dma_start(g1_sb, g1.unsqueeze(1))
                b1_sb = singles.tile([P, 1], F32)
                nc.scalar.dma_start(b1_sb, b1.unsqueeze(1))
                bhg_sb = singles.tile([P, HC], F32)
                nc.scalar.dma_start(bhg_sb, bhg.rearrange("(o p) -> p o", p=P))
                bo_sb = singles.tile([P, D], BF16)
                nc.gpsimd.dma_start(bo_sb, bo.unsqueeze(0).to_broadcast((P, D)))

                wqk_sb = singles.tile([P, KD, QK], FP8)
                nc.scalar.dma_start(wqk_sb, wqk.rearrange("(o p) q -> p o q", p=P))

                # persistent big tensors
                vsb = persist.tile([P, NSK, H], FP8, tag="vsb")  # v [j, h]
                kT = persist.tile([P, S], BF16, tag="kT")
                qT = persist.tile([P, SQ], BF16, tag="qT")
                ntq = persist.tile([P, KD, SQ], FP8, tag="ntq")  # query cols

                # ---------- Phase 0/1: LN stats + normalize + projections --
                GT = 4
                NGRP = NSK // GT
                with ExitStack() as ph1:
                    xbfp = ph1.enter_context(tc.tile_pool(name="xbfp", bufs=4))
                    statp = ph1.enter_context(tc.tile_pool(name="statp", bufs=4))
                    work = ph1.enter_context(tc.tile_pool(name="work", bufs=3))
                    nrmp = ph1.enter_context(tc.tile_pool(name="nrmp",
                                                          bufs=GT + 1))
                    qwork = ph1.enter_context(tc.tile_pool(name="qwork", bufs=3))
                    ps_tr = ph1.enter_context(
                        tc.tile_pool(name="ps_tr", bufs=2, space="PSUM"))
                    ntk = ph1.enter_context(
                        tc.tile_pool(name="ntkp", bufs=1)).tile(
                            [P, KD, S], FP8, tag="ntk")

                    mva = singles.tile([P, NSK, 2], F32)
                    rstd = singles.tile([P, NSK], F32)
                    nmu = singles.tile([P, NSK], F32)

                    def emit_stats(t):
                        xbf = xbfp.tile([P, D], BF16, tag="xbf")
                        nc.gpsimd.dma_start(xbf, xk[t * P:(t + 1) * P, :])
                        stats = statp.tile([P, 2, 6], F32, tag="st")
                        for i in range(2):
                            nc.vector.bn_stats(
                                out=stats[:, i, :],
                                in_=xbf[:, i * 512:(i + 1) * 512])
                        nc.vector.bn_aggr(out=mva[:, t, :], in_=stats)

                    def emit_rstd(lo, hi):
                        """batched rstd/-mu*rstd for tiles [lo, hi)."""
                        nc.scalar.activation(
                            out=rstd[:, lo:hi], in_=mva[:, lo:hi, 1],
                            func=mybir.ActivationFunctionType.Sqrt,
                            bias=eps_sb, scale=1.0)
                        nc.vector.reciprocal(out=rstd[:, lo:hi],
                                             in_=rstd[:, lo:hi])
                        nc.vector.tensor_mul(nmu[:, lo:hi], mva[:, lo:hi, 0],
                                             rstd[:, lo:hi])
                        nc.scalar.mul(nmu[:, lo:hi], nmu[:, lo:hi], -1.0)

                    whv_sb = ph1.enter_context(
                        tc.tile_pool(name="whvp", bufs=1)).tile(
                            [P, KD, H], FP8, tag="whv")
                    nc.scalar.dma_start(
                        whv_sb, whv.rearrange("(o p) h -> p o h", p=P))
                    bhv_sb = ph1.enter_context(
                        tc.tile_pool(name="bhvp", bufs=1)).tile(
                            [P, H], BF16, tag="bhv")
                    nc.gpsimd.dma_start(
                        bhv_sb, bhv.unsqueeze(0).to_broadcast((P, H)))
                    vwork = ph1.enter_context(tc.tile_pool(name="vwork", bufs=3))

                    def v_proj_tile(st):
                        """v rows for key tile st -> vsb[:, st, :]."""
                        for hb in range(HB):
                            ps = ps_mm.tile([P, NB], F32, tag="mm")
                            for k in range(0, KD, DR):
                                nc.tensor.matmul(
                                    ps, ntk[:, k:k + DR, st * P:(st + 1) * P],
                                    whv_sb[:, k:k + DR, hb * NB:(hb + 1) * NB],
                                    start=(k == 0), stop=(k == KD - DR),
                                    perf_mode=PM)
                            vtmp = vwork.tile([P, NB], BF16, tag="vtmp")
                            nc.vector.tensor_add(
                                out=vtmp, in0=ps,
                                in1=bhv_sb[:, hb * NB:(hb + 1) * NB])
                            nc.scalar.activation(
                                out=vsb[:, st, hb * NB:(hb + 1) * NB],
                                in_=vtmp,
                                func=mybir.ActivationFunctionType.Silu)

                    def qk_proj_block(sb):
                        """kT (and qT when sb < IB) for 512-col block sb."""
                        ps = ps_mm.tile([P, NB], F32, tag="mm")
                        for k in range(0, KD, DR):
                            nc.tensor.matmul(
                                ps, wqk_sb[:, k:k + DR, :],
                                ntk[:, k:k + DR, sb * NB:(sb + 1) * NB],
                                start=(k == 0), stop=(k == KD - DR),
                                perf_mode=PM)
                        tmp = qwork.tile([P, NB], BF16, tag="qtmp")
                        nc.scalar.activation(
                            out=tmp, in_=ps,
                            func=mybir.ActivationFunctionType.Silu,
                            bias=bqk_sb, scale=1.0)
                        nc.vector.tensor_scalar(
                            out=kT[:, sb * NB:(sb + 1) * NB],
                            in0=tmp, scalar1=g1_sb, scalar2=b1_sb,
                            op0=mybir.AluOpType.mult,
                            op1=mybir.AluOpType.add)
                        if sb < IB:
                            nc.vector.tensor_scalar(
                                out=qT[:, sb * NB:(sb + 1) * NB],
                                in0=tmp, scalar1=g0_sb, scalar2=b0_sb,
                                op0=mybir.AluOpType.mult,
                                op1=mybir.AluOpType.add)

                    # stats for the first half of tiles, then its rstd batch
                    HALF = NSK // 2
                    for t in range(HALF):
                        emit_stats(t)
                    emit_rstd(0, HALF)

                    for g in range(NGRP):
                        if g == NGRP // 2:
                            emit_rstd(HALF, NSK)
                        nrms = []
                        for tt in range(GT):
                            t = g * GT + tt
                            x_t = work.tile([P, D], F32, tag="xt")
                            nc.sync.dma_start(x_t, xk[t * P:(t + 1) * P, :])
                            nrm = nrmp.tile([P, D], BF16, tag="nrm",
                                            name=f"nrm{tt}")
                            nc.vector.tensor_scalar(
                                out=nrm, in0=x_t,
                                scalar1=rstd[:, t:t + 1],
                                scalar2=nmu[:, t:t + 1],
                                op0=mybir.AluOpType.mult,
                                op1=mybir.AluOpType.add)
                            nrms.append(nrm)
                            # interleave second-half stats into first half
                            if g < NGRP // 2:
                                emit_stats(HALF + g * GT + tt)
                        for k in range(KD):
                            pst = ps_tr.tile([P, GT, P], BF16, tag="pst")
                            for tt in range(GT):
                                nc.tensor.transpose(
                                    pst[:, tt, :],
                                    nrms[tt][:, k * P:(k + 1) * P], ident)
                            if k % 2 == 0:
                                nc.scalar.copy(
                                    out=ntk[:, k, g * GT * P:(g + 1) * GT * P],
                                    in_=pst)
                            else:
                                nc.vector.tensor_copy(
                                    out=ntk[:, k, g * GT * P:(g + 1) * GT * P],
                                    in_=pst)
                        # projections for the 4 tiles just transposed
                        for tt in range(GT):
                            v_proj_tile(g * GT + tt)
                        qk_proj_block(g)  # 8 groups == 8 kT blocks
                    nc.sync.dma_start(ntq, ntk[:, :, 0:SQ])

                # ---------- Phase 3: attention + gating + out-proj ----------
                with ExitStack() as ph3:
                    wp3 = ph3.enter_context(tc.tile_pool(name="wp3", bufs=1))
                    whg_sb = wp3.tile([P, KD, H], FP8, tag="whg")
                    nc.scalar.dma_start(
                        whg_sb, whg.rearrange("(o p) h -> p o h", p=P))
                    wo_sb = wp3.tile([P, HC, D], FP8, tag="wo")
                    nc.scalar.dma_start(
                        wo_sb, wo.rearrange("(o p) d -> p o d", p=P))
                    bt_sb = wp3.tile([P, 2, WT], BF16, tag="bt")
                    nc.scalar.dma_start(bt_sb[:, 0, :], bt0)
                    nc.scalar.dma_start(bt_sb[:, 1, :], bt1)

                    a2pool = ph3.enter_context(tc.tile_pool(name="a2p", bufs=2))
                    s1pool = ph3.enter_context(tc.tile_pool(name="s1p", bufs=2))
                    gtpool = ph3.enter_context(tc.tile_pool(name="gtp", bufs=2))
                    gopool = ph3.enter_context(tc.tile_pool(name="gop", bufs=2))
                    pspool = ph3.enter_context(tc.tile_pool(name="psp", bufs=2))
                    opool = ph3.enter_context(tc.tile_pool(name="op", bufs=2))
                    ps_sim = ph3.enter_context(
                        tc.tile_pool(name="ps_sim", bufs=2, space="PSUM"))
                    ps_gate = ph3.enter_context(
                        tc.tile_pool(name="ps_gate", bufs=2, space="PSUM"))
                    ps_acc = ph3.enter_context(
                        tc.tile_pool(name="ps_acc", bufs=2, space="PSUM"))

                    def emit_sim(ib, j):
                        """sim psum -> s1 = psum + bias -> attn2 = relu(s1)^2."""
                        ps = ps_sim.tile([P, NB], F32, tag="sim")
                        grp = 0 if j < JH else 1
                        jl = j - JH * grp
                        m0 = ib * NB - jl * P + (SQ - P)
                        nc.tensor.matmul(
                            ps, kT[:, j * P:(j + 1) * P],
                            qT[:, ib * NB:(ib + 1) * NB],
                            start=True, stop=True)
                        s1 = s1pool.tile([P, NB], BF16, tag="s1")
                        nc.vector.tensor_add(
                            out=s1, in0=ps, in1=bt_sb[:, grp, m0:m0 + NB])
                        nc.vector.scalar_tensor_tensor(
                            out=attn2s[ib % 2][:, j, :], in0=s1, scalar=0.0,
                            in1=s1,
                            op0=mybir.AluOpType.max,
                            op1=mybir.AluOpType.mult)

                    attn2s = [a2pool.tile([P, JC, NB], FP8, tag="attn2",
                                          name=f"attn2_{i}") for i in range(2)]
                    for j in range(JC):
                        emit_sim(0, j)
                    for ib in range(IB):
                        attn2 = attn2s[ib % 2]
                        goT = gopool.tile([P, HC, NB], FP8, tag="goT")
                        for hc in range(HC):
                            # gate psum [h, i]
                            gps = ps_gate.tile([P, NB], F32, tag="g")
                            for k in range(0, KD, DR):
                                nc.tensor.matmul(
                                    gps,
                                    whg_sb[:, k:k + DR, hc * P:(hc + 1) * P],
                                    ntq[:, k:k + DR, ib * NB:(ib + 1) * NB],
                                    start=(k == 0), stop=(k == KD - DR),
                                    perf_mode=PM)
                            gt = gtpool.tile([P, NB], BF16, tag="gt")
                            nc.scalar.activation(
                                out=gt, in_=gps,
                                func=mybir.ActivationFunctionType.Silu,
                                bias=bhg_sb[:, hc:hc + 1], scale=1.0)
                            # attention accumulation psum [h, i]
                            pacc = ps_acc.tile([P, NB], F32, tag="pacc")
                            for j in range(0, JC, DR):
                                nc.tensor.matmul(
                                    pacc,
                                    vsb[:, j:j + DR, hc * P:(hc + 1) * P],
                                    attn2[:, j:j + DR, :],
                                    start=(j == 0), stop=(j == JC - DR),
                                    perf_mode=PM)
                            nc.vector.tensor_mul(goT[:, hc, :], pacc, gt)
                            # interleave next i-block's sim pipeline
                            if ib + 1 < IB:
                                emit_sim(ib + 1, 2 * hc)
                                emit_sim(ib + 1, 2 * hc + 1)

                        # --- out projection + bias + residual ---
                        for isub in range(ISUB):
                            i0 = ib * NB + isub * P
                            xt = opool.tile([P, D], F32, tag="xres")
                            nc.sync.dma_start(xt, xk[i0:i0 + P, :])
                            for db in range(DB):
                                ps = ps_mm.tile([P, NB], F32, tag="mm")
                                for hc in range(0, HC, DR):
                                    nc.tensor.matmul(
                                        ps, goT[:, hc:hc + DR,
                                                isub * P:(isub + 1) * P],
                                        wo_sb[:, hc:hc + DR,
                                              db * NB:(db + 1) * NB],
                                        start=(hc == 0), stop=(hc == HC - DR),
                                        perf_mode=PM)
                                ot = opool.tile([P, NB], F32, tag="ot")
                                nc.vector.scalar_tensor_tensor(
                                    out=ot, in0=ps, scalar=ATTN_DESCALE,
                                    in1=bo_sb[:, db * NB:(db + 1) * NB],
                                    op0=mybir.AluOpType.mult,
                                    op1=mybir.AluOpType.add)
                                nc.gpsimd.tensor_add(
                                    ot, ot, xt[:, db * NB:(db + 1) * NB])
                                nc.sync.dma_start(
                                    out[i0:i0 + P, db * NB:(db + 1) * NB], ot)

    _split_excess_waits(nc)
    return nc


def _t5_bias_vec(rel_emb, S_, D_):
    """bv[r + S_-1] = bias for rel = k_pos - q_pos = r, scaled sqrt(D)/S."""
    r = np.arange(-(S_ - 1), S_, dtype=np.int64)
    n = (-r).astype(np.int64)
    nb = NUM_BUCKETS // 2
    me = nb // 2
    ret = (n < 0).astype(np.int64) * nb
    na = np.abs(n)
    val_large = me + (
        np.log(np.maximum(na, 1).astype(np.float32) / me)
        / math.log(MAX_DIST / me) * (nb - me)).astype(np.int64)
    val_large = np.minimum(val_large, nb - 1)
    bucket = ret + np.where(na < me, na, val_large)
    return (rel_emb[bucket, 0].astype(np.float64)
            * (float(D_) ** 0.5) / float(S_)).astype(np.float32)


def make_core_inputs(inputs, S_=S, SQ_=None, D_=D, QK_=QK, H_=H,
                     n_cores=N_CORES):
    """Build per-core in_maps from the full (unsharded) input dict."""
    if SQ_ is None:
        SQ_ = S_ // 2
    bf = ml_dtypes.bfloat16
    f8 = ml_dtypes.float8_e4m3fn
    x = np.asarray(inputs["x"], np.float32)
    Wh = np.asarray(inputs["Wh"], np.float64)
    bh = np.asarray(inputs["bh"], np.float64)
    Wqk = np.asarray(inputs["Wqk"], np.float64)
    bqk_ = np.asarray(inputs["bqk"], np.float64)
    osg = np.asarray(inputs["os_gamma"], np.float32)
    osb = np.asarray(inputs["os_beta"], np.float32)
    Wo = np.asarray(inputs["Wo"], np.float32)
    bo_ = np.asarray(inputs["bo"], np.float32)
    rel_emb = np.asarray(inputs["rel_emb"], np.float32)
    lng_ = np.asarray(inputs["ln_g"], np.float64)
    lnb_ = np.asarray(inputs["ln_b"], np.float64)

    # fold LN affine into the projection weights (exact algebra)
    Whf = lng_[:, None] * Wh
    bhf = bh + lnb_ @ Wh
    Wqkf = lng_[:, None] * Wqk
    bqkf = bqk_ + lnb_ @ Wqk

    bv = _t5_bias_vec(rel_emb, S_, D_)
    WT_ = (SQ_ - P) + SQ_
    halves = S_ // SQ_

    shared = dict(
        whv=np.ascontiguousarray(Whf[:, :H_]).astype(np.float32).astype(f8),
        whg=np.ascontiguousarray(Whf[:, H_:]).astype(np.float32).astype(f8),
        wqk=np.ascontiguousarray(Wqkf).astype(np.float32).astype(f8),
        wo=np.ascontiguousarray(Wo).astype(f8),
        bqk=bqkf.astype(np.float32),
        g0=(osg[0] * BT_SCALE / S_).astype(np.float32),
        b0=(osb[0] * BT_SCALE / S_).astype(np.float32),
        g1=osg[1].astype(np.float32),
        b1=osb[1].astype(np.float32),
        bhv=np.ascontiguousarray(bhf[:H_]).astype(np.float32),
        bhg=np.ascontiguousarray(bhf[H_:]).astype(np.float32),
        bo=bo_,
    )

    pp = np.arange(P)[:, None]
    cc = np.arange(WT_)[None, :]
    in_maps = []
    for c in range(n_cores):
        b = c // halves
        off = (c % halves) * SQ_
        other = SQ_ - off
        # reordered keys: own query half first
        xc = np.concatenate([x[b, off:off + SQ_], x[b, other:other + SQ_]],
                            axis=0)
        m = dict(shared)
        m["xk"] = np.ascontiguousarray(xc)
        # banded bias tables: bt_g[p, c] = bv[p - c + (SQ-P) + offj_g - offq
        #                                     + (S-1)] * BT_SCALE
        for gname, offj in (("bt0", off), ("bt1", other)):
            idx = pp - cc + (SQ_ - P) + offj - off + (S_ - 1)
            m[gname] = (bv[idx] * BT_SCALE).astype(bf)
        in_maps.append(m)
    return in_maps


def run_with_results(inputs, trace=False):
    key = (S, S // 2, D, QK, H)
    if key not in _NC_CACHE:
        _NC_CACHE[key] = build_gau_nc(*key)
    nc = _NC_CACHE[key]
    in_maps = make_core_inputs(inputs)
    res = run_bass_kernel_spmd(nc, in_maps, core_ids=list(range(N_CORES)),
                               trace=trace)
    SQ_ = S // 2
    halves = S // SQ_
    out = np.empty((B, S, D), np.float32)
    for c in range(N_CORES):
        b = c // halves
        off = (c % halves) * SQ_
        out[b, off:off + SQ_, :] = res.results[c]["out"]
    return out, res


def kernel(**inputs):
    return run_with_results(inputs, trace=False)[0]


# revision 30
# speedup vs baseline: 15.4043x; 15.3959x over previous
"""GAU (Gated Attention Unit) fused kernel for Trainium2, SPMD over 8 NeuronCores.

Sharding: data-parallel over batch (B=4) x query-sequence-halves (2) = 8 cores.
Each core computes the full GAU for its (batch, query-half).

v3 design (vs the DRAM-spill baseline):
  - Host folds ln_g/ln_b into Wh/Wqk (exact: (n*g+b)@W = n@(g.*W) + b@W).
  - Pair dedup: each core LayerNorms/projects only its OWN key half (which
    is also its query half, so query slicing is compile-time under SPMD);
    v and kT for the other half arrive via pair AllGather collectives
    (DRAM bounce buffers, groups [[0,1],[2,3],[4,5],[6,7]]).  The v
    exchange is chunked (tiles 0-7 / 8-11 / 12-15) so it overlaps the
    tail of the projection phase.  AllGather concat is group-rank order,
    so the T5 bias uses two banded tables with GLOBAL half offsets.
  - Two-pass LayerNorm: bn_stats from bf16 cast-DMA loads, batched Sqrt +
    reciprocal (2-4 act-table switches total, vs 2 per tile when
    Sqrt/Silu interleave).  Normalize runs on DVE (tensor_scalar with
    per-partition rstd/-mu*rstd scalars).
  - v ([j,h] fp8) is SBUF-resident (8MB) -- no per-i-block DRAM reload.
  - gate is computed on the fly per (i-block, h-chunk) in the attention
    phase; its psum is [h,i] so bhg folds into the Act silu bias.
  - Attention phase is software-pipelined: sim matmuls + bias/relu^2 for
    i-block N+1 are interleaved into the attn@v h-chunk loop of i-block N
    (attn2 double-buffered), keeping DVE work under PE matmuls.
"""

import math
import os
import sys

for _p in ("/opt/trn_rl_repo", "/root/.axon_site/_ro/trn_rl_repo"):
    if os.path.isdir(_p) and _p not in sys.path:
        sys.path.append(_p)

import numpy as np
import ml_dtypes

import concourse.bass as bass
import concourse.tile as tile
from concourse import mybir
from concourse.bass_utils import run_bass_kernel_spmd
from concourse.masks import make_identity

# Problem dims (hardcoded per spec)
B, S, D, QK, H = 4, 4096, 1024, 128, 2048
NUM_BUCKETS, MAX_DIST = 32, 128
LN_EPS = 1e-5
N_CORES = 8

P = 128
NB = 512  # free-dim block for matmuls

BF16 = mybir.dt.bfloat16
FP8 = mybir.dt.float8e4
F32 = mybir.dt.float32

AQ = 32.0   # q fp8 prescale
AK = 32.0   # k fp8 prescale
BT_SCALE = 1024.0                    # bias-table prescale (s1 = 1024*(sim+bias)/S)
SIM_DESCALE = BT_SCALE / (S * AQ * AK)
ATTN_DESCALE = 1.0 / (BT_SCALE * BT_SCALE)

_NC_CACHE = {}


def _split_excess_waits(nc, max_waits=1):
    """This container's walrus rejects instructions carrying more than one
    sem wait ("Too many sync wait commands").  Move excess waits onto
    same-engine nops inserted immediately before the instruction — engine
    FIFO order makes that semantically identical."""
    f = nc.m.functions[0]
    for bb in list(f.blocks):
        il = list(bb.instructions)
        out = []
        changed = False
        for inst in il:
            si = inst.sync_info
            if si is not None and si.on_wait and len(si.on_wait) > max_waits:
                waits = list(si.on_wait)
                moved, keep = waits[:-max_waits], waits[-max_waits:]
                si.on_wait = keep
                for w in moved:
                    eng = nc.engines[inst.engine]
                    cur_bb = nc.cur_bb.bb
                    n_before = len(cur_bb.instructions)
                    nop = eng.nop()
                    # pop the freshly appended nop from wherever it landed
                    tail = list(cur_bb.instructions)
                    assert tail[-1] is nop.ins and len(tail) == n_before + 1
                    cur_bb.instructions = tail[:-1]
                    nsi = nop.ins.sync_info
                    if nsi is None:
                        nop.ins.sync_info = mybir.SyncInfo(
                            on_wait=[w], on_update=[])
                    else:
                        nsi.on_wait = [w]
                    out.append(nop.ins)
                changed = True
            out.append(inst)
        if changed:
            bb.instructions = out


def _install_drain_wait_split():
    """The walrus build in this container rejects >1 sem wait on the Tile
    epilogue Drain ("Too many sync wait commands").  Split the extra waits
    onto explicit SP nops (they only need to precede the final barrier)."""
    from concourse.vector_clock import ScopedClock

    if getattr(tile.TileContext, "_drain_split_installed", False):
        return

    def _patched(self, tick_clock, wait_clock):
        drain_inst = self.nc.sync.drain()
        wait_clock.add_sem_waits(
            drain_inst.ins, ScopedClock({None: tick_clock.global_clock}))
        si = drain_inst.ins.sync_info
        if si is not None and si.on_wait and len(si.on_wait) > 1:
            extra = list(si.on_wait)[1:]
            si.on_wait = [si.on_wait[0]]
            for w in extra:
                nop = self.nc.sync.nop()
                nsi = nop.ins.sync_info
                if nsi is None:
                    nop.ins.sync_info = mybir.SyncInfo(on_wait=[w], on_update=[])
                else:
                    nsi.on_wait = [w]
        self.nc.all_engine_barrier()
        assert self.sems is not None
        popped = self.nc._tile_sem_poison_stack.pop()
        assert popped is self._sem_poison
        self.nc.clear_and_free_semaphores(list(self.sems.allocated().values()))
        self.nc.all_engine_barrier()

    tile.TileContext._drain_and_barrier = _patched
    tile.TileContext._drain_split_installed = True


_install_drain_wait_split()


def build_gau_nc(S=S, SQ=S // 2, D=D, QK=QK, H=H, reps=1, use_dr=True,
                 n_cores=N_CORES):
    DR = 2 if use_dr else 1
    PM = mybir.MatmulPerfMode.DoubleRow if use_dr else None
    assert D % P == 0 and H % P == 0 and S % NB == 0 and SQ % NB == 0
    assert QK == P
    KD = D // P      # d chunks (8)
    NSK = S // P     # key-side seq tiles (32)
    SBK = S // NB    # key-side 512-blocks (8)
    IB = SQ // NB    # query-side 512-blocks (4)
    HC = H // P      # h 128-chunks (16)
    HB = H // NB     # h 512-blocks (4)
    JC = S // P      # j chunks (32)
    JH = JC // 2     # j tiles per half group (16)
    DB = D // NB     # output d blocks (2)
    ISUB = NB // P   # i subtiles per i-block (4)
    WT = (SQ - P) + SQ   # per-group bias table width (3968)
    NSK_LOC = SQ // P    # own-half key tiles this core computes (16)
    SBK_LOC = SQ // NB   # own-half kT 512-blocks (4)
    GROUPS = [[2 * i, 2 * i + 1] for i in range(n_cores // 2)]

    nc = bass.Bass("TRN2", target_bir_lowering=False, debug=False,
                   num_devices=n_cores)

    # ---- DRAM I/O ----
    xk = nc.dram_tensor("xk", [SQ, D], F32, kind="ExternalInput").ap()
    whv = nc.dram_tensor("whv", [D, H], FP8, kind="ExternalInput").ap()
    whg = nc.dram_tensor("whg", [D, H], FP8, kind="ExternalInput").ap()
    wqk = nc.dram_tensor("wqk", [D, QK], FP8, kind="ExternalInput").ap()
    wo = nc.dram_tensor("wo", [H, D], FP8, kind="ExternalInput").ap()
    bqk = nc.dram_tensor("bqk", [QK], F32, kind="ExternalInput").ap()
    g0 = nc.dram_tensor("g0", [QK], F32, kind="ExternalInput").ap()  # gamma0*AQ
    b0 = nc.dram_tensor("b0", [QK], F32, kind="ExternalInput").ap()  # beta0*AQ
    g1 = nc.dram_tensor("g1", [QK], F32, kind="ExternalInput").ap()  # gamma1*AK
    b1 = nc.dram_tensor("b1", [QK], F32, kind="ExternalInput").ap()  # beta1*AK
    bhv = nc.dram_tensor("bhv", [H], F32, kind="ExternalInput").ap()
    bhg = nc.dram_tensor("bhg", [H], F32, kind="ExternalInput").ap()
    bo = nc.dram_tensor("bo", [D], F32, kind="ExternalInput").ap()
    bt0 = nc.dram_tensor("bt0", [P, WT], BF16, kind="ExternalInput").ap()
    bt1 = nc.dram_tensor("bt1", [P, WT], BF16, kind="ExternalInput").ap()
    out = nc.dram_tensor("out", [SQ, D], F32, kind="ExternalOutput").ap()

    with tile.TileContext(nc) as tc:
        for _rep in range(reps):
            from contextlib import ExitStack

            with ExitStack() as outer:
                singles = outer.enter_context(tc.tile_pool(name="singles", bufs=1))
                persist = outer.enter_context(tc.tile_pool(name="persist", bufs=1))
                ps_mm = outer.enter_context(
                    tc.tile_pool(name="ps_mm", bufs=2, space="PSUM"))

                ident = singles.tile([P, P], BF16)
                make_identity(nc, ident)

                eps_sb = singles.tile([P, 1], F32)
                nc.vector.memset(eps_sb, LN_EPS)

                # small parameter tiles
                bqk_sb = singles.tile([P, 1], F32)
                nc.scalar.dma_start(bqk_sb, bqk.unsqueeze(1))
                g0_sb = singles.tile([P, 1], F32)
                nc.scalar.dma_start(g0_sb, g0.unsqueeze(1))
                b0_sb = singles.tile([P, 1], F32)
                nc.scalar.dma_start(b0_sb, b0.unsqueeze(1))
                g1_sb = singles.tile([P, 1], F32)
                nc.scalar.dma_start(g1_sb, g1.unsqueeze(1))
                b1_sb = singles.tile([P, 1], F32)
                nc.scalar.dma_start(b1_sb, b1.unsqueeze(1))
                bhg_sb = singles.tile([P, HC], F32)
                nc.scalar.dma_start(bhg_sb, bhg.rearrange("(o p) -> p o", p=P))
                bo_sb = singles.tile([P, D], BF16)
                nc.gpsimd.dma_start(bo_sb, bo.unsqueeze(0).to_broadcast((P, D)))

                wqk_sb = singles.tile([P, KD, QK], FP8)
                nc.scalar.dma_start(wqk_sb, wqk.rearrange("(o p) q -> p o q", p=P))

                # persistent big tensors
                bt_sb = persist.tile([P, 2, WT], BF16, tag="bt")
                nc.scalar.dma_start(bt_sb[:, 0, :], bt0)
                nc.scalar.dma_start(bt_sb[:, 1, :], bt1)
                vsb = persist.tile([P, NSK, H], FP8, tag="vsb")  # v [j, h]
                kT = persist.tile([P, S], BF16, tag="kT")
                qT = persist.tile([P, SQ], BF16, tag="qT")
                # normed^T for the OWN half only == the query columns
                ntk = persist.tile([P, KD, SQ], FP8, tag="ntk")
                # DRAM bounce buffers for the pair AllGather of v / kT
                dramp = outer.enter_context(
                    tc.tile_pool(name="dramp", bufs=1, space="DRAM"))
                # v exchanged in 3 chunks (tiles 0-7, 8-11, 12-15) so the
                # AllGather overlaps the tail of phase 1
                VCH = [(0, NSK_LOC // 2),
                       (NSK_LOC // 2, 3 * NSK_LOC // 4),
                       (3 * NSK_LOC // 4, NSK_LOC)]
                vins = [dramp.tile([(t1 - t0) * P, H], FP8, tag=f"vin{i}",
                                   name=f"vin{i}")
                        for i, (t0, t1) in enumerate(VCH)]
                vouts = [dramp.tile([2 * (t1 - t0) * P, H], FP8,
                                    tag=f"vout{i}", name=f"vout{i}")
                         for i, (t0, t1) in enumerate(VCH)]
                # kT exchanged in 2 chunks so the sim pipeline starts early
                KS = SQ // 2
                kins = [dramp.tile([P, KS], BF16, tag=f"kin{i}",
                                   name=f"kin{i}") for i in range(2)]
                kouts = [dramp.tile([2, P, KS], BF16, tag=f"kout{i}",
                                    name=f"kout{i}") for i in range(2)]

                # ---------- Phase 0/1: LN stats + normalize + projections --
                GT = 4
                NGRP = NSK_LOC // GT
                with ExitStack() as ph1:
                    xbfp = ph1.enter_context(tc.tile_pool(name="xbfp", bufs=4))
                    statp = ph1.enter_context(tc.tile_pool(name="statp", bufs=4))
                    work = ph1.enter_context(tc.tile_pool(name="work", bufs=3))
                    nrmp = ph1.enter_context(tc.tile_pool(name="nrmp",
                                                          bufs=GT + 1))
                    qwork = ph1.enter_context(tc.tile_pool(name="qwork", bufs=3))
                    ps_tr = ph1.enter_context(
                        tc.tile_pool(name="ps_tr", bufs=2, space="PSUM"))
                    mva = singles.tile([P, NSK_LOC, 2], F32)
                    rstd = singles.tile([P, NSK_LOC], F32)
                    nmu = singles.tile([P, NSK_LOC], F32)

                    def emit_stats(t):
                        xbf = xbfp.tile([P, D], BF16, tag="xbf")
                        nc.gpsimd.dma_start(xbf, xk[t * P:(t + 1) * P, :])
                        stats = statp.tile([P, 2, 6], F32, tag="st")
                        for i in range(2):
                            nc.vector.bn_stats(
                                out=stats[:, i, :],
                                in_=xbf[:, i * 512:(i + 1) * 512])
                        nc.vector.bn_aggr(out=mva[:, t, :], in_=stats)

                    def emit_rstd(lo, hi):
                        """batched rstd/-mu*rstd for tiles [lo, hi)."""
                        nc.scalar.activation(
                            out=rstd[:, lo:hi], in_=mva[:, lo:hi, 1],
                            func=mybir.ActivationFunctionType.Sqrt,
                            bias=eps_sb, scale=1.0)
                        nc.vector.reciprocal(out=rstd[:, lo:hi],
                                             in_=rstd[:, lo:hi])
                        nc.vector.tensor_mul(nmu[:, lo:hi], mva[:, lo:hi, 0],
                                             rstd[:, lo:hi])
                        nc.scalar.mul(nmu[:, lo:hi], nmu[:, lo:hi], -1.0)

                    whv_sb = ph1.enter_context(
                        tc.tile_pool(name="whvp", bufs=1)).tile(
                            [P, KD, H], FP8, tag="whv")
                    nc.scalar.dma_start(
                        whv_sb, whv.rearrange("(o p) h -> p o h", p=P))
                    bhv_sb = ph1.enter_context(
                        tc.tile_pool(name="bhvp", bufs=1)).tile(
                            [P, H], BF16, tag="bhv")
                    nc.gpsimd.dma_start(
                        bhv_sb, bhv.unsqueeze(0).to_broadcast((P, H)))
                    vwork = ph1.enter_context(tc.tile_pool(name="vwork", bufs=3))

                    vrowp = ph1.enter_context(tc.tile_pool(name="vrowp",
                                                           bufs=2))

                    def v_proj_tile(st):
                        """v rows for own-half key tile st -> DRAM bounce."""
                        vrow = vrowp.tile([P, H], FP8, tag="vrow")
                        for hb in range(HB):
                            ps = ps_mm.tile([P, NB], F32, tag="mm")
                            for k in range(0, KD, DR):
                                nc.tensor.matmul(
                                    ps, ntk[:, k:k + DR, st * P:(st + 1) * P],
                                    whv_sb[:, k:k + DR, hb * NB:(hb + 1) * NB],
                                    start=(k == 0), stop=(k == KD - DR),
                                    perf_mode=PM)
                            vtmp = vwork.tile([P, NB], BF16, tag="vtmp")
                            nc.vector.tensor_add(
                                out=vtmp, in0=ps,
                                in1=bhv_sb[:, hb * NB:(hb + 1) * NB])
                            nc.scalar.activation(
                                out=vrow[:, hb * NB:(hb + 1) * NB],
                                in_=vtmp,
                                func=mybir.ActivationFunctionType.Silu)
                        ci = 0 if st < VCH[0][1] else (1 if st < VCH[1][1]
                                                       else 2)
                        r0 = (st - VCH[ci][0]) * P
                        nc.sync.dma_start(vins[ci][r0:r0 + P, :], vrow)

                    kTl = ph1.enter_context(
                        tc.tile_pool(name="kTlp", bufs=1)).tile(
                            [P, SQ], BF16, tag="kTl")

                    def qk_proj_block(sb):
                        """own-half kT block + qT block for 512-col block sb."""
                        ps = ps_mm.tile([P, NB], F32, tag="mm")
                        for k in range(0, KD, DR):
                            nc.tensor.matmul(
                                ps, wqk_sb[:, k:k + DR, :],
                                ntk[:, k:k + DR, sb * NB:(sb + 1) * NB],
                                start=(k == 0), stop=(k == KD - DR),
                                perf_mode=PM)
                        tmp = qwork.tile([P, NB], BF16, tag="qtmp")
                        nc.scalar.activation(
                            out=tmp, in_=ps,
                            func=mybir.ActivationFunctionType.Silu,
                            bias=bqk_sb, scale=1.0)
                        nc.vector.tensor_scalar(
                            out=kTl[:, sb * NB:(sb + 1) * NB],
                            in0=tmp, scalar1=g1_sb, scalar2=b1_sb,
                            op0=mybir.AluOpType.mult,
                            op1=mybir.AluOpType.add)
                        nc.vector.tensor_scalar(
                            out=qT[:, sb * NB:(sb + 1) * NB],
                            in0=tmp, scalar1=g0_sb, scalar2=b0_sb,
                            op0=mybir.AluOpType.mult,
                            op1=mybir.AluOpType.add)

                    # stats for the first half of tiles, then its rstd batch
                    # (no split when there are too few groups to interleave)
                    HALF = NSK_LOC // 2 if NGRP >= 2 else NSK_LOC
                    for t in range(HALF):
                        emit_stats(t)
                    emit_rstd(0, HALF)

                    def emit_k_exchange(ci):
                        nc.sync.dma_start(kins[ci], kTl[:, ci * KS:(ci + 1) * KS])
                        nc.gpsimd.collective_compute(
                            "AllGather", mybir.AluOpType.bypass,
                            replica_groups=GROUPS,
                            ins=[kins[ci][:]], outs=[kouts[ci][:]])
                        for r in range(2):
                            nc.sync.dma_start(
                                kT[:, r * SQ + ci * KS:r * SQ + (ci + 1) * KS],
                                kouts[ci][r])

                    def emit_v_exchange(ci):
                        t0, t1 = VCH[ci]
                        nt = t1 - t0
                        nc.gpsimd.collective_compute(
                            "AllGather", mybir.AluOpType.bypass,
                            replica_groups=GROUPS,
                            ins=[vins[ci][:]], outs=[vouts[ci][:]])
                        for r in range(2):
                            half = r * NSK_LOC
                            nc.scalar.dma_start(
                                vsb[:, half + t0:half + t1, :],
                                vouts[ci][r * nt * P:(r + 1) * nt * P, :]
                                .rearrange("(o p) h -> p o h", p=P))

                    next_vch = [0]
                    for g in range(NGRP):
                        if HALF < NSK_LOC and g == NGRP // 2:
                            emit_rstd(HALF, NSK_LOC)
                        nrms = []
                        for tt in range(GT):
                            t = g * GT + tt
                            x_t = work.tile([P, D], F32, tag="xt")
                            nc.sync.dma_start(x_t, xk[t * P:(t + 1) * P, :])
                            nrm = nrmp.tile([P, D], BF16, tag="nrm",
                                            name=f"nrm{tt}")
                            nc.vector.tensor_scalar(
                                out=nrm, in0=x_t,
                                scalar1=rstd[:, t:t + 1],
                                scalar2=nmu[:, t:t + 1],
                                op0=mybir.AluOpType.mult,
                                op1=mybir.AluOpType.add)
                            nrms.append(nrm)
                            # interleave second-half stats into first half
                            if HALF + g * GT + tt < NSK_LOC and g < NGRP // 2:
                                emit_stats(HALF + g * GT + tt)
                        for k in range(KD):
                            pst = ps_tr.tile([P, GT, P], BF16, tag="pst")
                            for tt in range(GT):
                                nc.tensor.transpose(
                                    pst[:, tt, :],
                                    nrms[tt][:, k * P:(k + 1) * P], ident)
                            nc.scalar.copy(
                                out=ntk[:, k, g * GT * P:(g + 1) * GT * P],
                                in_=pst)
                        # projections for the 4 tiles just transposed
                        for tt in range(GT):
                            v_proj_tile(g * GT + tt)
                        qk_proj_block(g)  # 4 groups == 4 own kT blocks
                        # kT chunk A gates the sim pipeline -> fire it
                        # before the same boundary's v chunk
                        if SBK_LOC >= 2 and (g + 1) * GT * P == KS:
                            emit_k_exchange(0)
                        while next_vch[0] < 2 and VCH[next_vch[0]][1] <= GT * (g + 1):
                            emit_v_exchange(next_vch[0])
                            next_vch[0] += 1
                    # remaining kT chunk, then any remaining v chunks
                    if SBK_LOC >= 2:
                        emit_k_exchange(1)
                    else:
                        emit_k_exchange(0)
                        emit_k_exchange(1)
                    for ci in range(next_vch[0], 3):
                        emit_v_exchange(ci)

                # ---------- Phase 3: attention + gating + out-proj ----------
                with ExitStack() as ph3:
                    wp3 = ph3.enter_context(tc.tile_pool(name="wp3", bufs=1))
                    whg_sb = wp3.tile([P, KD, H], FP8, tag="whg")
                    wo_sb = wp3.tile([P, HC, D], FP8, tag="wo")

                    a2pool = ph3.enter_context(tc.tile_pool(name="a2p", bufs=2))
                    s1pool = ph3.enter_context(tc.tile_pool(name="s1p", bufs=2))
                    gtpool = ph3.enter_context(tc.tile_pool(name="gtp", bufs=2))
                    gopool = ph3.enter_context(tc.tile_pool(name="gop", bufs=2))
                    pspool = ph3.enter_context(tc.tile_pool(name="psp", bufs=2))
                    opool = ph3.enter_context(tc.tile_pool(name="op", bufs=2))
                    ps_sim = ph3.enter_context(
                        tc.tile_pool(name="ps_sim", bufs=2, space="PSUM"))
                    ps_gate = ph3.enter_context(
                        tc.tile_pool(name="ps_gate", bufs=2, space="PSUM"))
                    ps_acc = ph3.enter_context(
                        tc.tile_pool(name="ps_acc", bufs=2, space="PSUM"))

                    def emit_sim(ib, j):
                        """sim psum -> s1 = psum + bias -> attn2 = relu(s1)^2."""
                        ps = ps_sim.tile([P, NB], F32, tag="sim")
                        grp = 0 if j < JH else 1
                        jl = j - JH * grp
                        m0 = ib * NB - jl * P + (SQ - P)
                        nc.tensor.matmul(
                            ps, kT[:, j * P:(j + 1) * P],
                            qT[:, ib * NB:(ib + 1) * NB],
                            start=True, stop=True)
                        s1 = s1pool.tile([P, NB], BF16, tag="s1")
                        nc.vector.tensor_add(
                            out=s1, in0=ps, in1=bt_sb[:, grp, m0:m0 + NB])
                        nc.vector.scalar_tensor_tensor(
                            out=attn2s[ib % 2][:, j, :], in0=s1, scalar=0.0,
                            in1=s1,
                            op0=mybir.AluOpType.max,
                            op1=mybir.AluOpType.mult)

                    JA = NSK_LOC // 2
                    J_ORDER = (list(range(0, JA))
                               + list(range(JH, JH + JA))
                               + list(range(JA, JH))
                               + list(range(JH + JA, JC)))
                    attn2s = [a2pool.tile([P, JC, NB], FP8, tag="attn2",
                                          name=f"attn2_{i}") for i in range(2)]
                    for j in J_ORDER:
                        emit_sim(0, j)
                        if j == 3:
                            # weight loads ride under the ib0 sim pipeline
                            nc.scalar.dma_start(
                                whg_sb, whg.rearrange("(o p) h -> p o h", p=P))
                            nc.scalar.dma_start(
                                wo_sb, wo.rearrange("(o p) d -> p o d", p=P))
                    for ib in range(IB):
                        attn2 = attn2s[ib % 2]
                        goT = gopool.tile([P, HC, NB], FP8, tag="goT")
                        for hc in range(HC):
                            # gate psum [h, i]
                            gps = ps_gate.tile([P, NB], F32, tag="g")
                            for k in range(0, KD, DR):
                                nc.tensor.matmul(
                                    gps,
                                    whg_sb[:, k:k + DR, hc * P:(hc + 1) * P],
                                    ntk[:, k:k + DR, ib * NB:(ib + 1) * NB],
                                    start=(k == 0), stop=(k == KD - DR),
                                    perf_mode=PM)
                            gt = gtpool.tile([P, NB], BF16, tag="gt")
                            nc.scalar.activation(
                                out=gt, in_=gps,
                                func=mybir.ActivationFunctionType.Silu,
                                bias=bhg_sb[:, hc:hc + 1], scale=1.0)
                            # attention accumulation psum [h, i]
                            pacc = ps_acc.tile([P, NB], F32, tag="pacc")
                            for idx in range(0, JC, DR):
                                j = J_ORDER[idx]
                                nc.tensor.matmul(
                                    pacc,
                                    vsb[:, j:j + DR, hc * P:(hc + 1) * P],
                                    attn2[:, j:j + DR, :],
                                    start=(idx == 0), stop=(idx == JC - DR),
                                    perf_mode=PM)
                            nc.vector.tensor_mul(goT[:, hc, :], pacc, gt)
                            # interleave next i-block's sim pipeline
                            if ib + 1 < IB:
                                emit_sim(ib + 1, J_ORDER[2 * hc])
                                emit_sim(ib + 1, J_ORDER[2 * hc + 1])

                        # --- out projection + bias + residual ---
                        for isub in range(ISUB):
                            i0 = ib * NB + isub * P
                            xt = opool.tile([P, D], F32, tag="xres")
                            nc.sync.dma_start(xt, xk[i0:i0 + P, :])
                            for db in range(DB):
                                ps = ps_mm.tile([P, NB], F32, tag="mm")
                                for hc in range(0, HC, DR):
                                    nc.tensor.matmul(
                                        ps, goT[:, hc:hc + DR,
                                                isub * P:(isub + 1) * P],
                                        wo_sb[:, hc:hc + DR,
                                              db * NB:(db + 1) * NB],
                                        start=(hc == 0), stop=(hc == HC - DR),
                                        perf_mode=PM)
                                ot = opool.tile([P, NB], F32, tag="ot")
                                nc.vector.scalar_tensor_tensor(
                                    out=ot, in0=ps, scalar=ATTN_DESCALE,
                                    in1=bo_sb[:, db * NB:(db + 1) * NB],
                                    op0=mybir.AluOpType.mult,
                                    op1=mybir.AluOpType.add)
                                nc.gpsimd.tensor_add(
                                    ot, ot, xt[:, db * NB:(db + 1) * NB])
                                nc.sync.dma_start(
                                    out[i0:i0 + P, db * NB:(db + 1) * NB], ot)

    _split_excess_waits(nc)
    return nc


def _t5_bias_vec(rel_emb, S_, D_):
    """bv[r + S_-1] = bias for rel = k_pos - q_pos = r, scaled sqrt(D)/S."""
    r = np.arange(-(S_ - 1), S_, dtype=np.int64)
    n = (-r).astype(np.int64)
    nb = NUM_BUCKETS // 2
    me = nb // 2
    ret = (n < 0).astype(np.int64) * nb
    na = np.abs(n)
    val_large = me + (
        np.log(np.maximum(na, 1).astype(np.float32) / me)
        / math.log(MAX_DIST / me) * (nb - me)).astype(np.int64)
    val_large = np.minimum(val_large, nb - 1)
    bucket = ret + np.where(na < me, na, val_large)
    return (rel_emb[bucket, 0].astype(np.float64)
            * (float(D_) ** 0.5) / float(S_)).astype(np.float32)


def make_core_inputs(inputs, S_=S, SQ_=None, D_=D, QK_=QK, H_=H,
                     n_cores=N_CORES):
    """Build per-core in_maps from the full (unsharded) input dict."""
    if SQ_ is None:
        SQ_ = S_ // 2
    bf = ml_dtypes.bfloat16
    f8 = ml_dtypes.float8_e4m3fn
    x = np.asarray(inputs["x"], np.float32)
    Wh = np.asarray(inputs["Wh"], np.float64)
    bh = np.asarray(inputs["bh"], np.float64)
    Wqk = np.asarray(inputs["Wqk"], np.float64)
    bqk_ = np.asarray(inputs["bqk"], np.float64)
    osg = np.asarray(inputs["os_gamma"], np.float32)
    osb = np.asarray(inputs["os_beta"], np.float32)
    Wo = np.asarray(inputs["Wo"], np.float32)
    bo_ = np.asarray(inputs["bo"], np.float32)
    rel_emb = np.asarray(inputs["rel_emb"], np.float32)
    lng_ = np.asarray(inputs["ln_g"], np.float64)
    lnb_ = np.asarray(inputs["ln_b"], np.float64)

    # fold LN affine into the projection weights (exact algebra)
    Whf = lng_[:, None] * Wh
    bhf = bh + lnb_ @ Wh
    Wqkf = lng_[:, None] * Wqk
    bqkf = bqk_ + lnb_ @ Wqk

    bv = _t5_bias_vec(rel_emb, S_, D_)
    WT_ = (SQ_ - P) + SQ_
    halves = S_ // SQ_

    shared = dict(
        whv=np.ascontiguousarray(Whf[:, :H_]).astype(np.float32).astype(f8),
        whg=np.ascontiguousarray(Whf[:, H_:]).astype(np.float32).astype(f8),
        wqk=np.ascontiguousarray(Wqkf).astype(np.float32).astype(f8),
        wo=np.ascontiguousarray(Wo).astype(f8),
        bqk=bqkf.astype(np.float32),
        g0=(osg[0] * BT_SCALE / S_).astype(np.float32),
        b0=(osb[0] * BT_SCALE / S_).astype(np.float32),
        g1=osg[1].astype(np.float32),
        b1=osb[1].astype(np.float32),
        bhv=np.ascontiguousarray(bhf[:H_]).astype(np.float32),
        bhg=np.ascontiguousarray(bhf[H_:]).astype(np.float32),
        bo=bo_,
    )

    pp = np.arange(P)[:, None]
    cc = np.arange(WT_)[None, :]
    in_maps = []
    for c in range(n_cores):
        b = c // halves
        off = (c % halves) * SQ_
        other = SQ_ - off
        # own query half only; the other key half arrives via AllGather in
        # GLOBAL order (group rank 0 = first half), so offj is global.
        m = dict(shared)
        m["xk"] = np.ascontiguousarray(x[b, off:off + SQ_])
        # banded bias tables: bt_g[p, c] = bv[p - c + (SQ-P) + offj_g - offq
        #                                     + (S-1)] * BT_SCALE
        for gname, offj in (("bt0", 0), ("bt1", SQ_)):
            idx = pp - cc + (SQ_ - P) + offj - off + (S_ - 1)
            m[gname] = (bv[idx] * BT_SCALE).astype(bf)
        in_maps.append(m)
    return in_maps


def run_with_results(inputs, trace=False):
    key = (S, S // 2, D, QK, H)
    if key not in _NC_CACHE:
        _NC_CACHE[key] = build_gau_nc(*key)
    nc = _NC_CACHE[key]
    in_maps = make_core_inputs(inputs)
    res = run_bass_kernel_spmd(nc, in_maps, core_ids=list(range(N_CORES)),
                               trace=trace)
    SQ_ = S // 2
    halves = S // SQ_
    out = np.empty((B, S, D), np.float32)
    for c in range(N_CORES):
        b = c // halves
        off = (c % halves) * SQ_
        out[b, off:off + SQ_, :] = res.results[c]["out"]
    return out, res


def kernel(**inputs):
    return run_with_results(inputs, trace=False)[0]
